# revision 25
# baseline (speedup 1.0000x reference)
"""Trainium2 Bass kernel for 3x3 valid conv (C_in=8, C_out=8, H=W=2048).

Strategy (spatial H-sharding across 8 cores):
  - Host splits x rows into 8 slabs of 256 output rows (+2 halo input rows)
    and packs each slab (fp16) into the exact SBUF layout the TensorE wants:
        xp[(ci, r), b, w] = slab[ci, h0(b) + r, w]
    for 19 row-blocks b (h0 = 14*b, last block 242), r = 0..15. Halo rows are
    duplicated host-side so every device load is a plain contiguous slice.
  - Three lhsT weight matrices (one per kw) of shape [K=128, M=112]:
        K = (ci, r), M = (co, j) with j = 0..13
        lhsT[kw][ci*16 + r, co*14 + j] = W[co, ci, r - j, kw]  (0 <= r-j <= 2)
  - Device per core: for each group of 4 blocks, one DMA loads
    [128, 4*2048] fp16; per block and per 512-wide column tile, 3
    accumulating matmuls (kw = 0,1,2; rhs shifted along the free axis)
    produce [112, 512] fp32 PSUM = out[(co, j), w]; PSUM tiles are copied
    (DVE/ACT alternating, fp32->fp16) into a [112, 4*2046] tile and stored
    with one DMA into op[(co, j), b, w]. Host scatters op back to
    (C, 2046, 2046) fp32.
"""

import numpy as np

import concourse.bass as bass
import concourse.mybir as mybir
import concourse.tile as tile
from concourse import bacc
from concourse.bass_utils import run_bass_kernel_spmd

# ---- problem geometry (hardcoded) ----
C = 8
H = 2048
W = 2048
KH = KW = 3
H_OUT = H - KH + 1   # 2046
W_OUT = W - KW + 1   # 2046
N_CORES = 8

ROWS_PER_CORE = 256          # output rows computed per core (core 7: 254 valid)
IN_ROWS = ROWS_PER_CORE + 2  # 258 input rows per core slab

J = 14                       # output rows per block
R = J + 2                    # 16 input rows per block
K = C * R                    # 128 contraction partitions
M = C * J                    # 112 output partitions
NBLK = 19                    # blocks per core
BLOCK_STARTS = [J * b for b in range(NBLK - 1)] + [ROWS_PER_CORE - J]
# h0(b) = 14*b for b<18, h0(18) = 242 (overlap-recompute tail)

COL_TILES = [(0, 512), (512, 512), (1024, 512), (1536, 510)]

IN_DT = mybir.dt.float16     # on-wire activation dtype
IN_NP = np.float16
OUT_DT = mybir.dt.float16    # on-wire output dtype (host upcasts)
OUT_NP = np.float16

GRP = 2                      # blocks per store DMA group
LOAD_GRP = 1                 # blocks per load DMA (divides into GRP groups)
Y_BUFS = 6
O_BUFS = 4


def build_nc(repeat: int = 1, mode: str = "full", grp: int = GRP,
             load_grp: int = LOAD_GRP, y_bufs: int = Y_BUFS, o_bufs: int = O_BUFS,
             load_eng: str = "pool"):
    do_mm = mode in ("full", "nocopy")
    do_copy = mode in ("full",)
    do_dma = mode in ("full", "nocopy", "dma")
    groups = [list(range(s, min(s + grp, NBLK))) for s in range(0, NBLK, grp)]
    nc = bacc.Bacc(
        "TRN2",
        target_bir_lowering=False,
        debug=False,
        num_devices=N_CORES,
    )
    xp = nc.dram_tensor("xp", [K, NBLK, W], IN_DT, kind="ExternalInput").ap()
    wts = nc.dram_tensor("wts", [KW, K, M], IN_DT, kind="ExternalInput").ap()
    op = nc.dram_tensor("op", [M, NBLK, W_OUT], OUT_DT, kind="ExternalOutput").ap()

    with tile.TileContext(nc) as tc:
        with (
            tc.tile_pool(name="wpool", bufs=1) as wpool,
            tc.tile_pool(name="ypool", bufs=y_bufs) as ypool,
            tc.tile_pool(name="opool", bufs=o_bufs) as opool,
            tc.tile_pool(name="pspool", bufs=8, space="PSUM") as pspool,
        ):
            wsb = wpool.tile([K, KW * M], IN_DT)
            for kw in range(KW):
                nc.sync.dma_start(wsb[:, kw * M:(kw + 1) * M], wts[kw])

            for rep_i in range(repeat):
                for blocks in groups:
                    g = len(blocks)
                    b0 = blocks[0]
                    # y is loaded in load_grp-block chunks for finer PE overlap
                    ys = []
                    for s in range(0, g, load_grp):
                        gl = min(load_grp, g - s)
                        yt = ypool.tile([K, gl * W], IN_DT, name="y", tag="y",
                                        padded_shape=[K, load_grp * W])
                        if do_dma:
                            if load_eng == "pool":
                                eng = nc.gpsimd
                            elif load_eng == "sp":
                                eng = nc.sync
                            else:  # alternate
                                eng = nc.gpsimd if (b0 + s) % (2 * load_grp) else nc.sync
                            eng.dma_start(yt[:], xp[:, b0 + s:b0 + s + gl, :])
                        ys.append(yt)

                    o = opool.tile([M, g * W_OUT], OUT_DT, name="o", tag="o",
                                   padded_shape=[M, grp * W_OUT])
                    for bi in range(g):
                        pss = []
                        for ti in range(len(COL_TILES)):
                            ps = pspool.tile([M, 512], mybir.dt.float32,
                                             name=f"ps{ti}", tag="ps")
                            pss.append(ps)
                        if do_mm:
                            y = ys[bi // load_grp]
                            yb = bi % load_grp
                            # kw-outer: consecutive MMs share the stationary side
                            for kw in range(KW):
                                for ti, (w0, n) in enumerate(COL_TILES):
                                    c0 = yb * W + w0 + kw
                                    nc.tensor.matmul(
                                        pss[ti][:, :n],
                                        lhsT=wsb[:, kw * M:(kw + 1) * M],
                                        rhs=y[:, c0:c0 + n],
                                        start=(kw == 0),
                                        stop=(kw == KW - 1),
                                    )
                        if do_copy:
                            for ti, (w0, n) in enumerate(COL_TILES):
                                dst = o[:, bi * W_OUT + w0:bi * W_OUT + w0 + n]
                                if ti % 2 == 0:
                                    nc.vector.tensor_copy(dst, pss[ti][:, :n])
                                else:
                                    nc.scalar.copy(dst, pss[ti][:, :n])
                    if not do_copy and do_dma:
                        # ablation modes: cheap writer so Tile allocates o
                        nc.vector.memset(o[:, :8], 0.0)
                    if do_dma:
                        nc.sync.dma_start(op[:, b0:b0 + g, :], o[:])

    nc.compile()
    return nc


def build_weight_lhst(weight: np.ndarray) -> np.ndarray:
    """weight: (C_out, C_in, 3, 3) fp32 -> (3, K, M) IN_NP."""
    wl = np.zeros((KW, K, M), np.float32)
    ci = np.arange(C)
    for kw in range(KW):
        for co in range(C):
            for j in range(J):
                for kh in range(KH):
                    r = j + kh
                    wl[kw, ci * R + r, co * J + j] = weight[co, :, kh, kw]
    return wl.astype(IN_NP)


def pack_core_input(slab: np.ndarray) -> np.ndarray:
    """slab: (C, IN_ROWS, W) fp16 -> xp (K, NBLK, W) fp16."""
    s0, s1, s2 = slab.strides
    # b = 0..17 uniform stride J; b = 18 special (h0 = 242)
    v = np.lib.stride_tricks.as_strided(
        slab, shape=(C, R, NBLK - 1, W), strides=(s0, s1, J * s1, s2)
    )
    xp = np.empty((C, R, NBLK, W), slab.dtype)
    xp[:, :, :NBLK - 1, :] = v
    xp[:, :, NBLK - 1, :] = slab[:, BLOCK_STARTS[-1]:BLOCK_STARTS[-1] + R, :]
    return xp.reshape(K, NBLK, W)


def unpack_core_output(op: np.ndarray) -> np.ndarray:
    """op: (M, NBLK, W_OUT) -> (C, ROWS_PER_CORE, W_OUT) float32."""
    op = op.reshape(C, J, NBLK, W_OUT)
    res = np.empty((C, ROWS_PER_CORE, W_OUT), np.float32)
    res[:, BLOCK_STARTS[-1]:, :] = op[:, :, NBLK - 1, :].astype(np.float32)
    res[:, :J * (NBLK - 1), :] = (
        op[:, :, :NBLK - 1, :].transpose(0, 2, 1, 3).reshape(C, J * (NBLK - 1), W_OUT)
    )
    return res


def shard_inputs(x: np.ndarray, weight: np.ndarray):
    xc = np.ascontiguousarray(x).astype(IN_NP)
    wl = build_weight_lhst(weight)
    in_maps = []
    for i in range(N_CORES):
        lo = i * ROWS_PER_CORE
        hi = min(lo + IN_ROWS, H)
        if hi - lo == IN_ROWS:
            slab = xc[:, lo:hi, :]
        else:
            slab = np.zeros((C, IN_ROWS, W), IN_NP)
            slab[:, :hi - lo, :] = xc[:, lo:hi, :]
        in_maps.append({"xp": pack_core_input(slab), "wts": wl})
    return in_maps


def unshard_output(results) -> np.ndarray:
    parts = []
    for i in range(N_CORES):
        rows = ROWS_PER_CORE if i < N_CORES - 1 else H_OUT - (N_CORES - 1) * ROWS_PER_CORE
        parts.append(unpack_core_output(results[i]["op"])[:, :rows, :])
    return np.concatenate(parts, axis=1)


_NC_CACHE = None


def _get_nc():
    global _NC_CACHE
    if _NC_CACHE is None:
        _NC_CACHE = build_nc()
    return _NC_CACHE


def run(inputs: dict, **spmd_kwargs):
    """Run the conv on 8 NeuronCores. Returns (full_output, BassKernelResults)."""
    in_maps = shard_inputs(np.asarray(inputs["x"]), np.asarray(inputs["weight"]))
    nc = _get_nc()
    res = run_bass_kernel_spmd(nc, in_maps, core_ids=list(range(N_CORES)), **spmd_kwargs)
    return unshard_output(res.results).astype(np.float32), res


def kernel(**inputs) -> np.ndarray:
    out, _ = run(inputs)
    return out
